# revision 9
# baseline (speedup 1.0000x reference)
import os
import sys
import tempfile

sys.path.insert(0, "/opt/trn_rl_repo")

from contextlib import ExitStack

import numpy as np

import concourse.bass as bass
import concourse.bacc as bacc
import concourse.mybir as mybir
import concourse.tile as tile
from concourse.bass_utils import run_bass_kernel_spmd

N = 50000
E = 1600000
IN = 128
H = 4
C = 16
HC = H * C
P = 128
NCORES = 8
NPC_PAD = 6272               # padded nodes per core (49 tiles of 128)
NT = NPC_PAD // P            # 49 dst tiles per core
NB = NCORES * NT             # 392 blocks of 128 nodes globally
NX = NB * P                  # 50176 padded node count
NG = NX // 512               # 98 projection groups of 4 blocks
RW = 72                      # table row: xp[64] | a_src[4] | a_dst[4] (bf16)

_cache = {}
_last_results = None


def build_program(dts):
    f32 = mybir.dt.float32
    bf16 = mybir.dt.bfloat16
    i32 = mybir.dt.int32
    X = mybir.AxisListType.X
    mult = mybir.AluOpType.mult
    add = mybir.AluOpType.add
    dmax = max(dts)

    nc = bacc.Bacc(None, target_bir_lowering=False, debug=False)
    xt_ext = nc.declare_dram_parameter("xt", [IN, NX], bf16, isOutput=False)
    wr_ext = nc.declare_dram_parameter("wrhs", [IN, RW], bf16, isOutput=False)
    bias_ext = nc.declare_dram_parameter("bias", [P, HC], f32, isOutput=False)
    idx_ext = nc.declare_dram_parameter("idx", [NPC_PAD, dmax + 1], i32,
                                        isOutput=False)
    out_ext = nc.declare_dram_parameter("out", [NPC_PAD, HC], f32, isOutput=True)
    xpa = nc.dram_tensor("xpa", [NX, RW], bf16)

    with tile.TileContext(nc) as tc, ExitStack() as ctx:
        singles = ctx.enter_context(tc.tile_pool(name="singles", bufs=1))
        wr_sb = singles.tile([IN, RW], bf16)
        nc.sync.dma_start(out=wr_sb[:], in_=wr_ext[:])
        bias_sb = singles.tile([P, HC], f32)
        nc.sync.dma_start(out=bias_sb[:], in_=bias_ext[:])
        neg_sb = singles.tile([1, 4], bf16)
        nc.vector.memset(neg_sb[:], -1e30)

        # ---- phase 1: xpa[n] = [x@W.T | x@Wa_src | x@Wa_dst]  (block-
        # interleaved row layout: newid g*512+b*128+p -> dram row g*512+p*4+b)
        with ExitStack() as p1:
            xbufs = p1.enter_context(tc.tile_pool(name="xbufs", bufs=3))
            psums = p1.enter_context(tc.tile_pool(name="psums", bufs=4,
                                                  space="PSUM"))
            obufs = p1.enter_context(tc.tile_pool(name="obufs", bufs=3))
            for g in range(NG):
                c0 = g * 512
                xtile = xbufs.tile([IN, 512], bf16)
                nc.sync.dma_start(out=xtile[:], in_=xt_ext[:, c0:c0 + 512])
                ps = psums.tile([P, 4 * RW], f32, space="PSUM")
                for b in range(4):
                    nc.tensor.matmul(out=ps[:, b * RW:(b + 1) * RW],
                                     lhsT=xtile[:, b * P:(b + 1) * P],
                                     rhs=wr_sb[:], start=True, stop=True)
                xa = obufs.tile([P, 4 * RW], bf16)
                nc.scalar.copy(out=xa[:], in_=ps[:])
                nc.sync.dma_start(
                    out=xpa[c0:c0 + 512, :].rearrange("(p q) w -> p (q w)",
                                                      p=P, q=4),
                    in_=xa[:])
        # padded slots point at a row whose a_src is -1e30 (exp -> 0)
        nc.sync.dma_start(out=xpa[NEG_ROW:NEG_ROW + 1, 64:68], in_=neg_sb[:])

        # ---- phase 2: per dst tile gather + softmax + aggregate
        gath = ctx.enter_context(tc.tile_pool(name="gath", bufs=2))
        small = ctx.enter_context(tc.tile_pool(name="small", bufs=3))
        for t in range(NT):
            D = dts[t]
            r0 = t * P
            idx_sb = small.tile([P, D + 1], i32)
            nc.sync.dma_start(out=idx_sb[:], in_=idx_ext[r0:r0 + P, :D + 1])

            xg = gath.tile([P, (D + 1) * RW], bf16)
            xg3 = xg[:].rearrange("p (d w) -> p d w", d=D + 1, w=RW)
            for d in range(D + 1):
                nc.gpsimd.indirect_dma_start(
                    out=xg3[:, d, :], out_offset=None, in_=xpa[:],
                    in_offset=bass.IndirectOffsetOnAxis(ap=idx_sb[:, d:d + 1],
                                                        axis=0))

            # e[p,d,h] = leaky(a_src[src] + a_dst[own]), pad slots -> -inf
            e = small.tile([P, D * H], f32)
            e3 = e[:].rearrange("p (d h) -> p d h", d=D, h=H)
            adst_b = xg3[:, D:D + 1, 68:72].to_broadcast([P, D, H])
            nc.vector.tensor_tensor(out=e3, in0=xg3[:, :D, 64:68], in1=adst_b,
                                    op=add)
            # leaky_relu(x) = max(0.2*x, x), fused on DVE
            nc.vector.scalar_tensor_tensor(out=e[:], in0=e[:], scalar=0.2,
                                           in1=e[:],
                                           op0=mult,
                                           op1=mybir.AluOpType.max)
            # expand exp(e) across channels on the scalar engine
            exb = gath.tile([P, D * HC], bf16)
            exb4 = exb[:].rearrange("p (d h c) -> p d h c", d=D, h=H, c=C)
            e4b = e[:].rearrange("p (d h one) -> p d h one", d=D, h=H,
                                 one=1).to_broadcast([P, D, H, C])
            nc.scalar.activation(out=exb4, in_=e4b,
                                 func=mybir.ActivationFunctionType.Exp)
            # softmax denominator from the c=0 lane of exb
            s = small.tile([P, H], f32)
            nc.vector.tensor_reduce(
                out=s[:],
                in_=exb[:].rearrange("p (d h c) -> p h c d", d=D, h=H,
                                     c=C)[:, :, 0:1, :],
                axis=X, op=add)
            sinv = small.tile([P, H], f32)
            nc.vector.reciprocal(out=sinv[:], in_=s[:])

            # msg = exp(e) * xp[src]; tree-reduce over d (bf16, 2x mode)
            msg = gath.tile([P, D * HC], bf16)
            msg3 = msg[:].rearrange("p (d w) -> p d w", d=D, w=HC)
            nc.vector.tensor_tensor(
                out=msg3[:, :, :], in0=xg3[:, :D, 0:64],
                in1=exb[:].rearrange("p (d w) -> p d w", d=D, w=HC), op=mult)
            n = D
            while n > 1:
                if n % 2:
                    nc.vector.tensor_tensor(out=msg3[:, n - 2, :],
                                            in0=msg3[:, n - 2, :],
                                            in1=msg3[:, n - 1, :], op=add)
                    n -= 1
                h = n // 2
                nc.vector.tensor_tensor(out=msg[:, :h * HC],
                                        in0=msg[:, :h * HC],
                                        in1=msg[:, h * HC:2 * h * HC], op=add)
                n = h

            outsb = small.tile([P, HC], f32)
            sinv_b = sinv[:].rearrange("p (h one) -> p h one",
                                       h=H, one=1).to_broadcast([P, H, C])
            nc.vector.tensor_tensor(
                out=outsb[:].rearrange("p (h c) -> p h c", h=H, c=C),
                in0=msg3[:, 0, :].rearrange("p (h c) -> p h c", h=H, c=C),
                in1=sinv_b, op=mult)
            nc.vector.tensor_tensor(out=outsb[:], in0=outsb[:], in1=bias_sb[:],
                                    op=add)
            nc.sync.dma_start(out=out_ext[r0:r0 + P, :], in_=outsb[:])

    nc.compile()
    return nc


def _rowof(n):
    return (n // 512) * 512 + (n % P) * 4 + ((n // P) % 4)


NEG_NEWID = NX - 1           # a guaranteed-fake node (only 50000 real)
ZERO_NEWID = NX - 2
NEG_ROW = _rowof(NEG_NEWID)
ZERO_ROW = _rowof(ZERO_NEWID)


def _preprocess(edge_index):
    src = edge_index[0].astype(np.int64)
    dst = edge_index[1].astype(np.int64)
    deg = np.bincount(dst, minlength=N)
    order = np.argsort(-deg, kind="stable")          # node ranks, deg desc

    # rank r (block k=r//128, lane p) -> newid (k%8)*6272 + (k//8)*128 + p
    r = np.arange(NX)
    k, p = r // P, r % P
    newid_of_rank = (k % NCORES) * NPC_PAD + (k // NCORES) * P + p
    node_of_newid = np.full(NX, -1, np.int64)
    node_of_newid[newid_of_rank[:N]] = order
    newid_of_node = np.empty(N, np.int64)
    newid_of_node[order] = newid_of_rank[:N]

    # per-tile degree bucket D_t = max degree within tile stratum (even, >=2)
    dts = []
    for j in range(NT):
        rank0 = j * P * NCORES
        d = int(deg[order[rank0]]) if rank0 < N else 0
        dts.append(max(2, d + (d % 2)))
    dmax = max(dts)

    # slot table: idx[newid, s] = table row of s-th in-neighbor
    dstn = newid_of_node[dst]
    srcr = _rowof(newid_of_node[src])
    osort = np.argsort(dstn, kind="stable")
    dstn_s, srcr_s = dstn[osort], srcr[osort]
    degn = np.bincount(dstn, minlength=NX)
    starts = np.zeros(NX + 1, np.int64)
    np.cumsum(degn, out=starts[1:])
    slot = np.arange(E, dtype=np.int64) - starts[dstn_s]
    idx_all = np.full((NX, dmax + 1), NEG_ROW, np.int32)
    idx_all[degn == 0, :] = ZERO_ROW                  # empty segments
    idx_all[dstn_s, slot] = srcr_s
    # own row (a_dst source) lives at column dts[tile] of each node's row
    nw = np.arange(NX)
    own_col = np.asarray(dts, np.int64)[(nw % NPC_PAD) // P]
    idx_all[nw, own_col] = _rowof(nw)
    return dts, idx_all, node_of_newid


def kernel(x, edge_index, W, att_src, att_dst, bias):
    x = np.asarray(x, np.float32)
    edge_index = np.asarray(edge_index)
    W = np.asarray(W, np.float32)
    att_src = np.asarray(att_src, np.float32)
    att_dst = np.asarray(att_dst, np.float32)
    bias = np.asarray(bias, np.float32)

    dts, idx_all, node_of_newid = _preprocess(edge_index)
    key = tuple(dts)
    if key not in _cache:
        _cache[key] = build_program(dts)
    nc = _cache[key]

    # x rows permuted to newid order, transposed, bf16
    x_perm = np.zeros((NX, IN), np.float32)
    valid = node_of_newid >= 0
    x_perm[valid] = x[node_of_newid[valid]]
    xt = np.ascontiguousarray(x_perm.T).astype(np.float32)

    # fused projection rhs: [W.T | Wa_src | Wa_dst]
    wa_src = np.einsum("ihc,hc->ih", W.T.reshape(IN, H, C), att_src)
    wa_dst = np.einsum("ihc,hc->ih", W.T.reshape(IN, H, C), att_dst)
    wrhs = np.concatenate([W.T, wa_src, wa_dst], axis=1)
    bias_rep = np.tile(bias.reshape(1, HC), (P, 1)).astype(np.float32)

    def bf16(a):
        import ml_dtypes
        return a.astype(ml_dtypes.bfloat16)

    in_maps = []
    for c in range(NCORES):
        lo, hi = c * NPC_PAD, (c + 1) * NPC_PAD
        in_maps.append({
            "xt": bf16(xt), "wrhs": bf16(wrhs), "bias": bias_rep,
            "idx": idx_all[lo:hi],
        })

    global _last_results
    tmpdir = None
    if os.environ.get("BASS_TRACE"):
        tmpdir = tempfile.mkdtemp(prefix="gat_trace_")
    _last_results = run_bass_kernel_spmd(nc, in_maps, list(range(NCORES)),
                                         tmpdir=tmpdir)
    res = _last_results.results
    arr = np.concatenate([np.asarray(res[c]["out"]) for c in range(NCORES)],
                         axis=0)
    out = np.zeros((N, HC), np.float32)
    out[node_of_newid[valid]] = arr[valid]
    return out
